# revision 1
# baseline (speedup 1.0000x reference)
"""Conv2d 3x3 VALID (NHWC x HWIO -> NHWC) on 8 Trainium2 NeuronCores.

Strategy: data-parallel over batch (2 images/core). Per core, the conv is an
implicit GEMM over the flattened H*W signal:

    out_flat[co, q] = sum_{r,s,ci} x_flat[ci, q + r*W + s] * w[r, s, ci, co]

with Cout=128 on PSUM partitions and 512-position moving windows (fp16
matmuls, fp32 PSUM accumulate, 1 cycle/row). The 9 taps are packed into five
K<=128 matmuls per window using SBUF-resident copies of the signal shifted by
1 and by W on partitions 64:128, so most matmuls use the full 128-row
contraction. Outputs at flat positions whose
column lands in {W-2, W-1} or row in {H-2, H-1} are garbage and are sliced
away host-side.

Self-contained: hardcodes shapes from the problem spec
  x: (16, 224, 224, 64) f32, w: (3, 3, 64, 128) f32 -> y: (16, 222, 222, 128).
"""
import contextlib
import os
import numpy as np

import concourse.bacc as bacc
import concourse.mybir as mybir
from concourse.tile import TileContext
from concourse.bass_utils import run_bass_kernel_spmd

N_CORES = 8
N_IMG = 2          # images per core
H = W = 224
CIN, COUT = 64, 128
L = H * W          # 50176 flat positions per image
Q = N_IMG * L      # 100352 output positions per core
WIN = 512          # moving-window width (one fp32 PSUM bank)
S = 4096           # slab positions kept in SBUF per iteration
MARGIN = 2 * W + 4
XT_W = Q + WIN     # zero-padded input width

VARIANT = os.environ.get("CONV_VARIANT", "v2")
OUT_DT = os.environ.get("CONV_OUT_DT", "f32")
IN_DT = os.environ.get("CONV_IN_DT", "f16")
A_BUFS = int(os.environ.get("CONV_A_BUFS", "3"))
PS_BUFS = int(os.environ.get("CONV_PS_BUFS", "8"))
O_BUFS = int(os.environ.get("CONV_O_BUFS", "8"))


def make_plan(variant):
    if variant == "v0":
        return [(0, r * W + s, 64, [(r, s), None]) for r in range(3) for s in range(3)]
    if variant == "v1":
        return ([(0, s, 128, [(0, s), (1, s)]) for s in range(3)]
                + [(0, 2 * W + s, 64, [(2, s), None]) for s in range(3)])
    if variant == "v2":
        return ([(0, r * W, 128, [(r, 0), (r, 1)]) for r in range(3)]
                + [(1, 2, 128, [(0, 2), (1, 2)]),
                   (0, 2 * W + 2, 64, [(2, 2), None])])
    raise ValueError(variant)


def build_nc(variant=VARIANT, out_dt=OUT_DT, s_pos=S,
             a_bufs=A_BUFS, ps_bufs=PS_BUFS, o_bufs=O_BUFS, repeat=1,
             order=os.environ.get("CONV_ORDER", "win"), in_dt=IN_DT):
    plan = make_plan(variant)
    n_mm = len(plan)
    ntl = 2 if variant == "v2" else 1
    f32 = mybir.dt.float32
    f32r = {"f32r": mybir.dt.float32r, "f16": mybir.dt.float16,
            "bf16": mybir.dt.bfloat16}[in_dt]
    out_mydt = f32 if out_dt == "f32" else mybir.dt.float16

    nc = bacc.Bacc("TRN2", target_bir_lowering=False, debug=False)
    xt = nc.declare_dram_parameter("xt", [CIN, XT_W], f32r, isOutput=False)
    wt = nc.declare_dram_parameter("wt", [n_mm, 128, COUT], f32r, isOutput=False)
    yt = nc.declare_dram_parameter("yt", [COUT, Q], out_mydt, isOutput=True)

    with TileContext(nc) as tc:
        with (
            tc.tile_pool(name="wpool", bufs=1) as wpool,
            tc.tile_pool(name="apool", bufs=a_bufs) as apool,
            tc.tile_pool(name="opool", bufs=o_bufs) as opool,
            tc.tile_pool(name="pspool", bufs=ps_bufs, space="PSUM") as pspool,
        ):
            w_sb = wpool.tile([128, n_mm * COUT], f32r)
            for i in range(n_mm):
                nc.sync.dma_start(out=w_sb[:, i * COUT:(i + 1) * COUT],
                                  in_=wt[i, :, :])

            n_slabs = (Q + s_pos - 1) // s_pos
            rep = 0
            # repeat>1 wraps the whole body in a HW loop purely for timing:
            # per-pass time = (T(repeat=N) - T(repeat=1)) / (N-1)
            loop_cm = tc.For_i(0, repeat, 1) if repeat > 1 \
                else contextlib.nullcontext()
            with loop_cm:
              for si in range(n_slabs):
                base = si * s_pos
                sh = min(s_pos, Q - base)
                tiles = [apool.tile([128, s_pos + MARGIN], f32r, tag=f"t{t}",
                                    name=f"tile{t}_{rep}_{si}")
                         for t in range(ntl)]
                nc.sync.dma_start(out=tiles[0][0:CIN, 0:sh + MARGIN],
                                  in_=xt[:, base:base + sh + MARGIN])
                if variant == "v1":
                    nc.sync.dma_start(out=tiles[0][CIN:128, 0:sh + 2],
                                      in_=tiles[0][0:CIN, W:W + sh + 2])
                elif variant == "v2":
                    nc.sync.dma_start(out=tiles[0][CIN:128, 0:2 * W + sh],
                                      in_=tiles[0][0:CIN, 1:2 * W + sh + 1])
                    nc.sync.dma_start(out=tiles[1][0:CIN, 0:sh + 2],
                                      in_=tiles[0][0:CIN, 0:sh + 2])
                    nc.sync.dma_start(out=tiles[1][CIN:128, 0:sh + 2],
                                      in_=tiles[0][0:CIN, W:W + sh + 2])

                if order == "win":
                    for q0 in range(0, sh, WIN):
                        acc = pspool.tile([128, WIN], f32)
                        for j, (t, off, kk, _) in enumerate(plan):
                            nc.tensor.matmul(
                                acc[:],
                                w_sb[0:kk, j * COUT:(j + 1) * COUT],
                                tiles[t][0:kk, off + q0: off + q0 + WIN],
                                start=(j == 0),
                                stop=(j == n_mm - 1),
                            )
                        st = opool.tile([128, WIN], out_mydt)
                        nc.vector.tensor_copy(st[:], acc[:])
                        nc.sync.dma_start(out=yt[:, base + q0: base + q0 + WIN],
                                          in_=st[:])
                else:  # tap-major: one weight load serves every window in slab
                    q0s = list(range(0, sh, WIN))
                    accs = [pspool.tile([128, WIN], f32,
                                        name=f"acc_{rep}_{si}_{qi}", tag="acc")
                            for qi in range(len(q0s))]
                    for j, (t, off, kk, _) in enumerate(plan):
                        for qi, q0 in enumerate(q0s):
                            nc.tensor.matmul(
                                accs[qi][:],
                                w_sb[0:kk, j * COUT:(j + 1) * COUT],
                                tiles[t][0:kk, off + q0: off + q0 + WIN],
                                start=(j == 0),
                                stop=(j == n_mm - 1),
                            )
                    for qi, q0 in enumerate(q0s):
                        st = opool.tile([128, WIN], out_mydt)
                        nc.vector.tensor_copy(st[:], accs[qi][:])
                        nc.sync.dma_start(out=yt[:, base + q0: base + q0 + WIN],
                                          in_=st[:])
    nc.compile()
    return nc


def pack_wt(w, variant=VARIANT):
    plan = make_plan(variant)
    wt = np.zeros((len(plan), 128, COUT), dtype=np_in_dt())
    for i, (_, _, _, taps) in enumerate(plan):
        (r0, s0), bot = taps
        wt[i, 0:CIN] = w[r0, s0]
        if bot is not None:
            r1, s1 = bot
            wt[i, CIN:128] = w[r1, s1]
    return wt


def np_in_dt(in_dt=IN_DT):
    if in_dt == "f16":
        return np.float16
    if in_dt == "bf16":
        import ml_dtypes
        return np.dtype(ml_dtypes.bfloat16)
    return np.float32


def prep_xt(xs, in_dt=IN_DT):
    """xs: (N_IMG, H, W, 64) f32 -> (64, XT_W) channel-major flattened + pad."""
    flat = np.ascontiguousarray(xs.transpose(3, 0, 1, 2)).reshape(CIN, N_IMG * L)
    out = np.zeros((CIN, XT_W), dtype=np_in_dt(in_dt))
    out[:, :flat.shape[1]] = flat
    return out


def post_yt(yt_arr):
    """(128, Q) -> (N_IMG, 222, 222, 128) f32."""
    y = np.asarray(yt_arr, dtype=np.float32).reshape(COUT, N_IMG, H, W)
    y = y[:, :, :H - 2, :W - 2]
    return np.ascontiguousarray(y.transpose(1, 2, 3, 0))


_NC_CACHE = {}


def _get_nc():
    key = (VARIANT, OUT_DT, S, A_BUFS, PS_BUFS, O_BUFS)
    if key not in _NC_CACHE:
        _NC_CACHE[key] = build_nc()
    return _NC_CACHE[key]


def make_in_maps(x, w):
    wt = pack_wt(w)
    return [{"xt": prep_xt(x[c * N_IMG:(c + 1) * N_IMG]), "wt": wt}
            for c in range(N_CORES)]


def kernel(x, w):
    x = np.asarray(x, dtype=np.float32)
    w = np.asarray(w, dtype=np.float32)
    nc = _get_nc()
    in_maps = make_in_maps(x, w)
    res = run_bass_kernel_spmd(nc, in_maps, list(range(N_CORES)))
    out = np.empty((N_CORES * N_IMG, H - 2, W - 2, COUT), dtype=np.float32)
    for c in range(N_CORES):
        out[c * N_IMG:(c + 1) * N_IMG] = post_yt(res.results[c]["yt"])
    return out



# revision 2
# speedup vs baseline: 1.6057x; 1.6057x over previous
"""Conv2d 3x3 VALID (NHWC x HWIO -> NHWC) on 8 Trainium2 NeuronCores.

Strategy: data-parallel over batch (2 images/core). Per core, an implicit GEMM
in a row-parity layout: SBUF partition p = ci + 64*(h mod 2), free index
q = n*(112*224) + (h//2)*224 + w. In this layout 6 of the 9 taps pack into
K=128 matmuls with no data movement (taps of two adjacent input rows align at
the same free offset), and the remaining 3+3 K=64 taps are issued to disjoint
PE row groups (partitions 0:64 vs 64:128) so pairs execute concurrently in
the 128x128 array. No SBUF->SBUF copy DMAs at all.

Per 512-wide window pair (even+odd output planes, 1024 outputs):
  acc_e = sum_s E_s.T @ xp[:, q0+s]  + Ce_s.T @ xp[0:64, q0+224+s]
  acc_o = sum_s O_s.T @ xp[:, q0+224+s] + Co_s.T @ xp[64:128, q0+s]
with E_s = [w[0,s]; w[1,s]], O_s = [w[1,s]; w[2,s]], Ce_s = w[2,s],
Co_s = w[0,s]. fp16 operands, fp32 PSUM, fp16 output (halves write traffic).
Garbage columns (w >= 222) and rows are sliced away host-side.

Self-contained: hardcodes shapes from the problem spec
  x: (16, 224, 224, 64) f32, w: (3, 3, 64, 128) f32 -> y: (16, 222, 222, 128).
"""
import contextlib
import os
import numpy as np

import concourse.bacc as bacc
import concourse.mybir as mybir
from concourse.tile import TileContext
from concourse.bass_utils import run_bass_kernel_spmd

N_CORES = 8
N_IMG = 2            # images per core
H = W = 224
CIN, COUT = 64, 128
HP = H // 2          # 112 half-rows per image
LP = HP * W          # 25088 positions per image per plane
QP = N_IMG * LP      # 50176 positions per plane per core
WIN = 512            # moving-window width (one fp32 PSUM bank)
MARGIN = W + 32      # windows read up to q0 + 224 + 2 + 511
XT_W = QP + MARGIN   # zero-padded input width

S = int(os.environ.get("CONV_S", "6144"))       # slab positions per iteration
A_BUFS = int(os.environ.get("CONV_A_BUFS", "3"))
PS_BUFS = int(os.environ.get("CONV_PS_BUFS", "8"))
O_BUFS = int(os.environ.get("CONV_O_BUFS", "8"))
GRP = int(os.environ.get("CONV_GRP", "3"))      # window-pairs per tap-major group


def build_nc(s_pos=S, a_bufs=A_BUFS, ps_bufs=PS_BUFS, o_bufs=O_BUFS,
             grp=GRP, repeat=1):
    f16 = mybir.dt.float16
    f32 = mybir.dt.float32

    nc = bacc.Bacc("TRN2", target_bir_lowering=False, debug=False)
    xt = nc.declare_dram_parameter("xt", [128, XT_W], f16, isOutput=False)
    wt = nc.declare_dram_parameter("wt", [128, 12 * COUT], f16, isOutput=False)
    yt = nc.declare_dram_parameter("yt", [COUT, 2 * QP], f16, isOutput=True)

    with TileContext(nc) as tc:
        with (
            tc.tile_pool(name="wpool", bufs=1) as wpool,
            tc.tile_pool(name="apool", bufs=a_bufs) as apool,
            tc.tile_pool(name="opool", bufs=o_bufs) as opool,
            tc.tile_pool(name="pspool", bufs=ps_bufs, space="PSUM") as pspool,
        ):
            w_sb = wpool.tile([128, 12 * COUT], f16)
            nc.sync.dma_start(out=w_sb[:], in_=wt[:, :])

            def wtile(i):  # stationary tile i (full 128 rows)
                return w_sb[:, i * COUT:(i + 1) * COUT]

            n_slabs = (QP + s_pos - 1) // s_pos
            loop_cm = tc.For_i(0, repeat, 1) if repeat > 1 \
                else contextlib.nullcontext()
            with loop_cm:
              for si in range(n_slabs):
                base = si * s_pos
                sh = min(s_pos, QP - base)
                tl = apool.tile([128, s_pos + MARGIN], f16, tag="x",
                                name=f"x_{si}")
                nc.sync.dma_start(out=tl[:, 0:sh + MARGIN],
                                  in_=xt[:, base:base + sh + MARGIN])

                q0s = list(range(0, sh, WIN))
                for gi in range(0, len(q0s), grp):
                    grp_q = q0s[gi:gi + grp]
                    acc_e = [pspool.tile([128, WIN], f32, tag="acc",
                                         name=f"ae_{si}_{gi}_{k}")
                             for k in range(len(grp_q))]
                    acc_o = [pspool.tile([128, WIN], f32, tag="acc",
                                         name=f"ao_{si}_{gi}_{k}")
                             for k in range(len(grp_q))]
                    # full-K taps, tap-major so weights stay loaded across
                    # the group's windows
                    for s in range(3):
                        for k, q0 in enumerate(grp_q):
                            nc.tensor.matmul(
                                acc_e[k][:], wtile(s),
                                tl[:, q0 + s: q0 + s + WIN],
                                start=(s == 0), stop=False)
                    for s in range(3):
                        for k, q0 in enumerate(grp_q):
                            nc.tensor.matmul(
                                acc_o[k][:], wtile(3 + s),
                                tl[:, q0 + W + s: q0 + W + s + WIN],
                                start=(s == 0), stop=False)
                    # K=64 leftover taps on disjoint row groups: Ce on
                    # partitions 0:64, Co on 64:128 -> concurrent pairs
                    for s in range(3):
                        for k, q0 in enumerate(grp_q):
                            nc.tensor.matmul(
                                acc_e[k][:],
                                w_sb[0:64, (6 + s) * COUT:(7 + s) * COUT],
                                tl[0:64, q0 + W + s: q0 + W + s + WIN],
                                start=False, stop=(s == 2))
                        for k, q0 in enumerate(grp_q):
                            nc.tensor.matmul(
                                acc_o[k][:],
                                w_sb[64:128, (9 + s) * COUT:(10 + s) * COUT],
                                tl[64:128, q0 + s: q0 + s + WIN],
                                start=False, stop=(s == 2))
                    for k, q0 in enumerate(grp_q):
                        for acc, off in ((acc_e[k], 0), (acc_o[k], QP)):
                            st = opool.tile([128, WIN], f16)
                            nc.vector.tensor_copy(st[:], acc[:])
                            nc.sync.dma_start(
                                out=yt[:, off + base + q0: off + base + q0 + WIN],
                                in_=st[:])
    nc.compile()
    return nc


def pack_wt(w):
    """w: (3,3,64,128) f32 -> (128, 12*128) f16 stationary tiles."""
    wt = np.zeros((128, 12 * COUT), dtype=np.float16)
    for s in range(3):
        wt[0:64, s * COUT:(s + 1) * COUT] = w[0, s]          # E_s rows 0:64
        wt[64:128, s * COUT:(s + 1) * COUT] = w[1, s]        # E_s rows 64:128
        wt[0:64, (3 + s) * COUT:(4 + s) * COUT] = w[1, s]    # O_s rows 0:64
        wt[64:128, (3 + s) * COUT:(4 + s) * COUT] = w[2, s]  # O_s rows 64:128
        wt[0:64, (6 + s) * COUT:(7 + s) * COUT] = w[2, s]    # Ce_s (par0)
        wt[64:128, (9 + s) * COUT:(10 + s) * COUT] = w[0, s] # Co_s (par1)
    return wt


def prep_xt(xs):
    """xs: (N_IMG, 224, 224, 64) f32 -> (128, XT_W) f16 parity layout."""
    t = xs.transpose(3, 0, 1, 2).reshape(CIN, N_IMG, HP, 2, W)
    out = np.zeros((128, XT_W), dtype=np.float16)
    out[0:CIN, :QP] = t[:, :, :, 0, :].reshape(CIN, QP)
    out[CIN:128, :QP] = t[:, :, :, 1, :].reshape(CIN, QP)
    return out


def post_yt(yt_arr):
    """(128, 2*QP) -> (N_IMG, 222, 222, 128) f32."""
    y = np.asarray(yt_arr, dtype=np.float32)
    oe = y[:, :QP].reshape(COUT, N_IMG, HP, W)
    oo = y[:, QP:].reshape(COUT, N_IMG, HP, W)
    out = np.empty((N_IMG, H - 2, W - 2, COUT), dtype=np.float32)
    out[:, 0::2] = oe[:, :, :111, :W - 2].transpose(1, 2, 3, 0)
    out[:, 1::2] = oo[:, :, :111, :W - 2].transpose(1, 2, 3, 0)
    return out


_NC_CACHE = {}


def _get_nc():
    key = (S, A_BUFS, PS_BUFS, O_BUFS, GRP)
    if key not in _NC_CACHE:
        _NC_CACHE[key] = build_nc()
    return _NC_CACHE[key]


def make_in_maps(x, w):
    wt = pack_wt(np.asarray(w, dtype=np.float32))
    return [{"xt": prep_xt(x[c * N_IMG:(c + 1) * N_IMG]), "wt": wt}
            for c in range(N_CORES)]


def kernel(x, w):
    x = np.asarray(x, dtype=np.float32)
    w = np.asarray(w, dtype=np.float32)
    nc = _get_nc()
    in_maps = make_in_maps(x, w)
    res = run_bass_kernel_spmd(nc, in_maps, list(range(N_CORES)))
    out = np.empty((N_CORES * N_IMG, H - 2, W - 2, COUT), dtype=np.float32)
    for c in range(N_CORES):
        out[c * N_IMG:(c + 1) * N_IMG] = post_yt(res.results[c]["yt"])
    return out
